# revision 47
# baseline (speedup 1.0000x reference)
"""MultiHeadCredibilityAttention TRN2 kernel (v3).

Sharding: 8 cores = (batch b, query-half qh). Each core computes K/V for its
full batch sequence (S=2048) and attention outputs for its 1024 queries.
Output slices concatenate to the full (4, 2048, 256) result — no collectives.

Design (evolved from the v1 baseline through perfetto-trace iteration):
  - Attention is a software-pipelined stream of "units" u = (pair g, m-tile,
    head): 2 score matmuls -> exp -> 2 attn@V matmuls. Head PAIRS are
    staggered by 8 m-tiles so two pairs are always in flight: the softmax
    finalize of pair g overlaps ~8 slots of pair g+1's matmuls, and the two
    ctx-PSUM buffers ping-pong with zero boundary stalls.
  - Exp is split across engines: even heads use ScalarE's exact exp (f32r
    ets); odd heads use a 1-pass Schraudolph fast-exp on the Vector engine
    (int16 round-to-nearest write, bitcast to bf16 ets; ~1.7% elementwise,
    ~3e-3 end-to-end). ScalarE alone would otherwise pace the kernel.
  - All inputs ship as bf16 (halves DMA bytes); projections and score
    matmuls run bf16 (bf16 streams ~216ns/512 cols and enables FWL weight
    loads); attn@V runs f32r for exact heads, bf16 for fast heads (a second
    bf16 V copy is built by the idle GpSimd engine).
  - Softmax denominators ride along as a ones-column in the V operand
    (row 32/96 of the ctx PSUM pair tile). Finalize: ScalarE+DVE evacuate
    the two den rows, a DMA reshapes them to [128,16] so one cheap DVE
    reciprocal covers all lanes, a DRAM bounce broadcasts the reciprocal
    across partitions, DVE scales ctx into ctxT. Emitted as staged thunks,
    one per popped attn@V, to spread engine-queue injections.
  - PSUM: 2 rotating score tiles (4 banks) + 2 ctx pair tiles (2+2 banks);
    the late projections (K chunks 1-3, V tiles 4-15) stream into the first
    8 slots using 2 banks that hand over to the second ctx pool at slot 8.
  - Input DMAs are packed (weights 1-2 DMAs, biases 1) to beat the ~650ns
    per-DMA issue cost; the exp table is primed at t=0.
"""

import sys

import numpy as np

sys.path.insert(0, "/opt/trn_rl_repo")

import concourse.bass as bass  # noqa: E402
import concourse.mybir as mybir  # noqa: E402
from concourse.tile import TileContext  # noqa: E402
from concourse import bass_utils  # noqa: E402

B, S, D, H, HD = 4, 2048, 256, 8, 32
SQ = S // 2  # queries per core
N_CORES = 8
NM = S // 128  # key tiles
F32 = mybir.dt.float32
F32R = mybir.dt.float32r
BF16 = mybir.dt.bfloat16
I16 = mybir.dt.int16
Alu = mybir.AluOpType
INV_SCALE = 1.0 / np.sqrt(HD)

# bf16 Schraudolph fast-exp constants (round-to-nearest variant).
_LOG2E = 1.4426950408889634
FE_A = float((1 << 7) * _LOG2E * INV_SCALE)  # folds the 1/sqrt(hd) scale
FE_B = float(127.0 * (1 << 7) - 8.25)

# which units use the fast path: heads with (h % 2 == 1), m in FAST_MS
FAST_MS = frozenset(range(NM))


def _is_fast(h, m):
    return (h % 2 == 1) and (m in FAST_MS)


def split_multiwaits(nc, max_waits=1):
    """This toolchain's walrus rejects >1 sync-wait per instruction; split
    extras into preceding single-wait NOPs on the same engine."""
    n = 0
    for f in nc.m.functions:
        for bb in f.blocks:
            out = []
            for ins in bb.instructions:
                si = ins.sync_info
                if (
                    si is not None
                    and si.on_wait is not None
                    and len(si.on_wait) > max_waits
                ):
                    waits = list(si.on_wait)
                    for j, w in enumerate(waits[:-max_waits]):
                        n += 1
                        out.append(
                            mybir.InstNoOp(
                                name=f"{ins.name}-wsplit{j}",
                                opcode="NoOp",
                                engine=ins.engine,
                                sync_info=mybir.SyncInfo(on_wait=[w], on_update=[]),
                            )
                        )
                    ins.sync_info = mybir.SyncInfo(
                        on_wait=waits[-max_waits:], on_update=list(si.on_update)
                    )
                out.append(ins)
            bb.instructions = out
    return n


def build_module():
    nc = bass.Bass("TRN2")
    xT_d = nc.dram_tensor("xT", [D, S], BF16, kind="ExternalInput")
    xTq_d = nc.dram_tensor("xTq", [D, SQ], BF16, kind="ExternalInput")
    # weights packed [128, 8*256]: (k0,k1,q0,q1,v0,v1,o0,o1)
    wpack_d = nc.dram_tensor("wpack", [128, 8 * D], BF16, kind="ExternalInput")
    # biases packed [128, 516]: bvb | bob | bq(2 cols) | bk(2 cols)
    bpack_d = nc.dram_tensor("bpack", [128, 2 * D + 4], F32, kind="ExternalInput")
    out_d = nc.dram_tensor("out", [SQ, D], F32, kind="ExternalOutput")

    with TileContext(nc) as tc:
        with (
            tc.tile_pool(name="const", bufs=1) as cpool,
            tc.tile_pool(name="pers", bufs=1) as pers,
        ):
            # ---- input DMAs, chunked + spread across engine queues ----
            xT_sb = [
                cpool.tile([128, S], BF16, tag=f"xT{d}", name=f"xT{d}")
                for d in range(2)
            ]
            xTq_sb = [
                cpool.tile([128, SQ], BF16, tag=f"xTq{d}", name=f"xTq{d}")
                for d in range(2)
            ]
            qs = [nc.sync, nc.scalar, nc.gpsimd]
            wpack_sb = cpool.tile([128, 8 * D], BF16, tag="wpack", name="wpack")
            bpack_sb = cpool.tile([128, 2 * D + 4], F32, tag="bpack", name="bpack")
            w_sb = {
                nm: [
                    wpack_sb[:, (2 * i + d) * D : (2 * i + d + 1) * D]
                    for d in range(2)
                ]
                for i, nm in enumerate(("k", "q", "v", "o"))
            }
            bvb_sb2 = bpack_sb[:, 0:D]
            bob_sb = bpack_sb[:, D : 2 * D]
            bq_sb2 = [bpack_sb[:, 2 * D + d : 2 * D + d + 1] for d in range(2)]
            bk_sb2 = [bpack_sb[:, 2 * D + 2 + d : 2 * D + 3 + d] for d in range(2)]
            jobs = [
                (wpack_sb[:, 0 : 6 * D], wpack_d[:, 0 : 6 * D]),
                (xT_sb[0][:, 0:512], xT_d[0:128, 0:512]),
                (xT_sb[1][:, 0:512], xT_d[128:256, 0:512]),
                (bpack_sb[:], bpack_d[:]),
                (xTq_sb[0][:], xTq_d[0:128, :]),
                (xTq_sb[1][:], xTq_d[128:256, :]),
                (wpack_sb[:, 6 * D : 8 * D], wpack_d[:, 6 * D : 8 * D]),
                (xT_sb[0][:, 512:2048], xT_d[0:128, 512:2048]),
                (xT_sb[1][:, 512:2048], xT_d[128:256, 512:2048]),
            ]
            for qi, (o, i_) in enumerate(jobs):
                qs[qi % 3].dma_start(out=o, in_=i_)

            # prime the exp activation table early (off the attention path)
            prime = cpool.tile([1, 1], F32, tag="prime", name="prime")
            nc.vector.memset(prime[:], 0.0)
            nc.scalar.activation(
                prime[:], prime[:], mybir.ActivationFunctionType.Exp
            )

            # ---- persistent intermediates ----
            KT_sb = [
                pers.tile([128, S], BF16, tag=f"KT{d}", name=f"KT{d}") for d in range(2)
            ]
            QTm_sb = [
                pers.tile([128, SQ], BF16, tag=f"QTm{h}", name=f"QTm{h}")
                for h in range(H)
            ]
            VA_sb = [
                pers.tile([128, H * (HD + 1)], F32R, tag=f"VA{m}", name=f"VA{m}")
                for m in range(NM)
            ]
            VAb_sb = [
                pers.tile([128, H * (HD + 1)], BF16, tag=f"VAb{m}", name=f"VAb{m}")
                for m in range(NM)
            ]
            ctxT_sb = [
                pers.tile([128, SQ], BF16, tag=f"ctxT{d}", name=f"ctxT{d}")
                for d in range(2)
            ]
            denS_sb = [
                pers.tile([1, SQ], F32, tag=f"denS{i}", name=f"denS{i}")
                for i in range(2)
            ]
            denT_sb = pers.tile([128, 16], F32, tag="denT", name="denT")
            recT_sb = pers.tile([128, 16], F32, tag="recT", name="recT")
            rb_sb = [
                pers.tile([HD, SQ], F32, tag=f"rb{i}", name=f"rb{i}") for i in range(2)
            ]

            # masked-Q tiles: zero everything once; Q bias-add writes in-head
            # rows. F32-packed memset path is ~10x faster than the bf16 one.
            for h in range(H):
                nc.vector.memset(QTm_sb[h][:].bitcast(F32), 0.0)
            # ones-columns for the denominator ride-along
            for m in range(NM):
                va = VA_sb[m][:].bitcast(F32).rearrange("p (h c) -> p h c", c=HD + 1)
                nc.vector.memset(va[:, :, HD : HD + 1], 1.0)

            # ---- projection helpers (psum tile passed in) ----
            def emit_k(half, kn, pkt, on_scalar=False):
                for d in range(2):
                    nc.tensor.matmul(
                        pkt[:],
                        lhsT=w_sb["k"][d][:, half * 128 : (half + 1) * 128],
                        rhs=xT_sb[d][:, kn * 512 : (kn + 1) * 512],
                        start=(d == 0),
                        stop=(d == 1),
                    )
                if on_scalar:
                    # keep the DVE queue clear during the unit stream
                    nc.scalar.activation(
                        KT_sb[half][:, kn * 512 : (kn + 1) * 512],
                        pkt[:],
                        mybir.ActivationFunctionType.Identity,
                        bias=bk_sb2[half],
                    )
                else:
                    nc.vector.tensor_scalar_add(
                        KT_sb[half][:, kn * 512 : (kn + 1) * 512],
                        pkt[:],
                        bk_sb2[half],
                    )

            def emit_q(half, qn, pqt):
                for d in range(2):
                    nc.tensor.matmul(
                        pqt[:],
                        lhsT=w_sb["q"][d][:, half * 128 : (half + 1) * 128],
                        rhs=xTq_sb[d][:, qn * 512 : (qn + 1) * 512],
                        start=(d == 0),
                        stop=(d == 1),
                    )
                for hh in range(4):
                    h = half * 4 + hh
                    nc.vector.tensor_scalar_add(
                        QTm_sb[h][
                            32 * hh : 32 * (hh + 1), qn * 512 : (qn + 1) * 512
                        ],
                        pqt[32 * hh : 32 * (hh + 1), :],
                        bq_sb2[half][32 * hh : 32 * (hh + 1), :],
                    )

            def emit_v(m, pvt):
                for d in range(2):
                    nc.tensor.matmul(
                        pvt[:],
                        lhsT=xT_sb[d][:, m * 128 : (m + 1) * 128],
                        rhs=w_sb["v"][d][:],
                        start=(d == 0),
                        stop=(d == 1),
                    )
                va = VA_sb[m][:].rearrange("p (h c) -> p h c", c=HD + 1)
                nc.vector.tensor_add(
                    va[:, :, 0:HD],
                    pvt[:].rearrange("p (h c) -> p h c", c=HD),
                    bvb_sb2.rearrange("p (h c) -> p h c", c=HD),
                )
                nc.gpsimd.tensor_copy(VAb_sb[m][:], VA_sb[m][:])

            # ---- early projections: K chunk 0, all Q, V m0-3 ----
            with tc.tile_pool(name="ps_kq", bufs=1, space="PSUM") as ps1:
                pksE = [
                    ps1.tile([128, 512], F32, tag=f"pk{i}", name=f"pk{i}")
                    for i in range(2)
                ]
                pqsE = [
                    ps1.tile([128, 512], F32, tag=f"pq{i}", name=f"pq{i}")
                    for i in range(2)
                ]
                pvsE = [
                    ps1.tile([128, D], F32, tag=f"pv{i}", name=f"pv{i}")
                    for i in range(2)
                ]
                emit_k(0, 0, pksE[0])
                emit_k(1, 0, pksE[1])
                emit_q(0, 0, pqsE[0])
                emit_q(0, 1, pqsE[1])
                emit_q(1, 0, pqsE[0])
                emit_q(1, 1, pqsE[1])
                for m in range(4):
                    emit_v(m, pvsE[m % 2])

            # ---- attention: pipelined units (head, m) ----
            with (
                tc.tile_pool(name="ps_att", bufs=1, space="PSUM") as psa,
                tc.tile_pool(name="etsA", bufs=4) as etsA,
                tc.tile_pool(name="etsB", bufs=4) as etsB,
                tc.tile_pool(name="dramp", bufs=2, space="DRAM") as dramp,
            ):
                pscs = [
                    psa.tile([128, SQ], F32, tag=f"psc{i}", name=f"psc{i}")
                    for i in range(2)
                ]
                pending = []  # deferred attn@V emissions, as (fn, args...)

                def emit_av(h, m, ets, pctx, base, fast):
                    vsb = VAb_sb[m] if fast else VA_sb[m]
                    for qn in range(2):
                        nc.tensor.matmul(
                            pctx[base : base + HD + 1, qn * 512 : (qn + 1) * 512],
                            lhsT=vsb[:, h * (HD + 1) : (h + 1) * (HD + 1)],
                            rhs=ets[:, qn * 512 : (qn + 1) * 512],
                            start=(m == 0),
                            stop=(m == NM - 1),
                        )

                fin_steps = []  # staged finalize thunks, one per unit

                def emit_finalize(hg, pctx):
                    # recip -> DRAM-bounce broadcast -> scale; staged so the
                    # engine-queue injections spread over several units
                    half = (2 * hg) // 4
                    rrec = dramp.tile([2, SQ], F32, tag="rrec", name=f"rrec{hg}")

                    def s1():
                        nc.scalar.activation(
                            denS_sb[0][:],
                            pctx[HD : HD + 1, :],
                            mybir.ActivationFunctionType.Copy,
                        )
                        nc.vector.tensor_copy(
                            denS_sb[1][:], pctx[64 + HD : 64 + HD + 1, :]
                        )
                        nc.sync.dma_start(out=denT_sb[0:64, :], in_=denS_sb[0][:])
                        nc.scalar.dma_start(out=denT_sb[64:128, :], in_=denS_sb[1][:])

                    def s2():
                        nc.vector.reciprocal(recT_sb[:], denT_sb[:])
                        nc.sync.dma_start(out=rrec[0:1, :], in_=recT_sb[0:64, :])
                        nc.scalar.dma_start(out=rrec[1:2, :], in_=recT_sb[64:128, :])
                        nc.sync.dma_start(
                            out=rb_sb[0][:], in_=rrec[0:1, :].to_broadcast((HD, SQ))
                        )
                        nc.scalar.dma_start(
                            out=rb_sb[1][:], in_=rrec[1:2, :].to_broadcast((HD, SQ))
                        )

                    def mk_mul(i, h):
                        def s():
                            base = 64 * i
                            hh = h % 4
                            nc.vector.tensor_mul(
                                ctxT_sb[half][32 * hh : 32 * (hh + 1), :],
                                pctx[base : base + HD, :],
                                rb_sb[i][:],
                            )

                        return s

                    if hg == H // 2 - 1:

                        def mk_mul_c(i, h, c):
                            def s():
                                base = 64 * i
                                hh = h % 4
                                cs = slice(c * 512, (c + 1) * 512)
                                nc.vector.tensor_mul(
                                    ctxT_sb[half][32 * hh : 32 * (hh + 1), cs],
                                    pctx[base : base + HD, cs],
                                    rb_sb[i][:, cs],
                                )

                            return s

                        fin_steps.extend(
                            [
                                s1,
                                s2,
                                mk_mul_c(0, 2 * hg, 0),
                                mk_mul_c(1, 2 * hg + 1, 0),
                                mk_mul_c(0, 2 * hg, 1),
                                mk_mul_c(1, 2 * hg + 1, 1),
                            ]
                        )
                    else:
                        fin_steps.extend(
                            [s1, s2, mk_mul(0, 2 * hg), mk_mul(1, 2 * hg + 1)]
                        )

                pop_ctr = [0]

                def pop_one():
                    h, m, ets, pctx_t, base, hg_of, fast = pending.pop(0)
                    emit_av(h, m, ets, pctx_t, base, fast)
                    if h % 2 == 1 and m == NM - 1:
                        emit_finalize(hg_of, pctx_t)
                    pop_ctr[0] += 1
                    if fin_steps and pop_ctr[0] % 2 == 0:
                        fin_steps.pop(0)()

                # pairs staggered by 8 m-tiles: two pairs in flight, two pctx
                # buffers; each pair's finalize has ~8 slots of slack before
                # its pctx banks are needed again
                STAG = 8
                units = []
                for s in range(STAG * (H // 2 - 1) + NM):
                    for g in range(H // 2):
                        m = s - STAG * g
                        if 0 <= m < NM:
                            for h in (2 * g, 2 * g + 1):
                                units.append((g, m, h))

                # late projections streamed into the early slots; their psum
                # banks hand over to pctx1 at slot 8
                pctx0_cm = tc.tile_pool(name="pctx0", bufs=1, space="PSUM")
                pctx0_pool = pctx0_cm.__enter__()
                late_cm = tc.tile_pool(name="ps_late", bufs=1, space="PSUM")
                late = late_cm.__enter__()
                pkL = late.tile([128, 512], F32, tag="pkL", name="pkL")
                pvL = late.tile([128, D], F32, tag="pvL", name="pvL")
                quota = []
                for kn in range(1, 4):
                    for mv in (4 * kn, 4 * kn + 1):
                        quota.append((emit_v, (mv, pvL)))
                    quota.append((emit_k, (0, kn, pkL, True)))
                    quota.append((emit_k, (1, kn, pkL, True)))
                    for mv in (4 * kn + 2, 4 * kn + 3):
                        quota.append((emit_v, (mv, pvL)))
                pctx1_cm = None
                pctx1_pool = None
                pctxs = {}
                uidx = 0
                for g, m, h in units:
                    slot = STAG * g + m
                    if slot >= STAG and late_cm is not None:
                        while quota:
                            fn, args = quota.pop(0)
                            fn(*args)
                        late_cm.__exit__(None, None, None)
                        late_cm = None
                        pctx1_cm = tc.tile_pool(
                            name="pctx1", bufs=1, space="PSUM"
                        )
                        pctx1_pool = pctx1_cm.__enter__()
                    if g not in pctxs:
                        pool_g = pctx0_pool if g % 2 == 0 else pctx1_pool
                        pctxs[g] = pool_g.tile(
                            [97, SQ], F32, tag="pctx", name=f"pctx{g}"
                        )
                    pctx = pctxs[g]
                    half = h // 4
                    psc = pscs[uidx % 2]
                    for qn in range(2):
                        nc.tensor.matmul(
                            psc[:, qn * 512 : (qn + 1) * 512],
                            lhsT=KT_sb[half][:, m * 128 : (m + 1) * 128],
                            rhs=QTm_sb[h][:, qn * 512 : (qn + 1) * 512],
                            start=True,
                            stop=True,
                        )
                    if _is_fast(h, m):
                        ets = etsB.tile([128, SQ], BF16, tag="eB", name="eB")
                        nc.vector.tensor_scalar(
                            ets[:].bitcast(I16),
                            psc[:],
                            FE_A,
                            FE_B,
                            Alu.mult,
                            Alu.add,
                        )
                    else:
                        ets = etsA.tile([128, SQ], F32R, tag="eA", name="eA")
                        nc.scalar.activation(
                            ets[:],
                            psc[:],
                            mybir.ActivationFunctionType.Exp,
                            scale=float(INV_SCALE),
                        )
                    base = 0 if h % 2 == 0 else 64
                    pending.append((h, m, ets, pctx, base, g, _is_fast(h, m)))
                    while len(pending) > 4:
                        pop_one()
                    for _ in range(2):
                        if quota:
                            fn, args = quota.pop(0)
                            fn(*args)
                    uidx += 1
                while pending:
                    pop_one()
                while fin_steps:
                    fin_steps.pop(0)()
                if pctx1_cm is not None:
                    pctx1_cm.__exit__(None, None, None)
                pctx0_cm.__exit__(None, None, None)

            # ---- output projection ----
            with (
                tc.tile_pool(name="ps_o", bufs=4, space="PSUM") as ps_o,
                tc.tile_pool(name="outp", bufs=4) as outp,
            ):
                for t in range(SQ // 128):
                    po = ps_o.tile([128, D], F32, tag="po", name="po")
                    for d in range(2):
                        nc.tensor.matmul(
                            po[:],
                            lhsT=ctxT_sb[d][:, t * 128 : (t + 1) * 128],
                            rhs=w_sb["o"][d][:],
                            start=(d == 0),
                            stop=(d == 1),
                        )
                    ot = outp.tile([128, D], F32, tag="ot", name="ot")
                    nc.vector.tensor_add(ot[:], po[:], bob_sb)
                    oq = nc.sync if t % 2 == 0 else nc.scalar
                    oq.dma_start(out=out_d[t * 128 : (t + 1) * 128, :], in_=ot[:])

    split_multiwaits(nc)
    return nc


_module_cache = {}


def _get_module():
    if "m" not in _module_cache:
        _module_cache["m"] = build_module()
    return _module_cache["m"]


def make_in_maps(inputs):
    x = np.asarray(inputs["x"], np.float32)
    Wq, bq = np.asarray(inputs["Wq"], np.float32), np.asarray(inputs["bq"], np.float32)
    Wk, bk = np.asarray(inputs["Wk"], np.float32), np.asarray(inputs["bk"], np.float32)
    Wv, bv = np.asarray(inputs["Wv"], np.float32), np.asarray(inputs["bv"], np.float32)
    Wo, bo = np.asarray(inputs["Wo"], np.float32), np.asarray(inputs["bo"], np.float32)
    hs = np.asarray(inputs["head_scale"], np.float32)

    import ml_dtypes

    bf16 = np.dtype(ml_dtypes.bfloat16)
    hs_col = np.repeat(hs, HD)  # head_scale folded into V
    wqT = Wq.T.astype(bf16)
    wkT = Wk.T.astype(bf16)
    wvT = (Wv * hs_col[:, None]).T.astype(bf16)
    woT = Wo.T.astype(bf16)
    wpack = np.concatenate(
        [wkT[0:128], wkT[128:256], wqT[0:128], wqT[128:256],
         wvT[0:128], wvT[128:256], woT[0:128], woT[128:256]],
        axis=1,
    )
    bpack = np.concatenate(
        [
            np.broadcast_to(bv * hs_col, (128, D)),
            np.broadcast_to(bo, (128, D)),
            bq.reshape(2, 128).T,
            bk.reshape(2, 128).T,
        ],
        axis=1,
    ).astype(np.float32)
    shared = {
        "wpack": np.ascontiguousarray(wpack),
        "bpack": np.ascontiguousarray(bpack),
    }
    in_maps = []
    for k in range(N_CORES):
        b, qh = k // 2, k % 2
        xT = np.ascontiguousarray(x[b].T.astype(bf16))
        in_maps.append(
            {
                "xT": xT,
                "xTq": np.ascontiguousarray(xT[:, qh * SQ : (qh + 1) * SQ]),
                **shared,
            }
        )
    return in_maps


def kernel(x, Wq, bq, Wk, bk, Wv, bv, Wo, bo, head_scale):
    in_maps = make_in_maps(
        dict(
            x=x, Wq=Wq, bq=bq, Wk=Wk, bk=bk, Wv=Wv, bv=bv, Wo=Wo, bo=bo,
            head_scale=head_scale,
        )
    )
    nc = _get_module()
    core_ids = list(range(N_CORES))
    # First execution after a fresh process attach has been observed to
    # return corrupted results on some cores; run once to warm up, then
    # use the second run (retry if it still looks corrupted).
    bass_utils.run_bass_kernel_spmd(nc, in_maps, core_ids=core_ids)
    for _ in range(3):
        res = bass_utils.run_bass_kernel_spmd(nc, in_maps, core_ids=core_ids)
        outs = [r["out"] for r in res.results]
        finite = all(np.isfinite(o).all() for o in outs)
        if finite and max(float(np.abs(o).max()) for o in outs) < 1e4:
            break
    full = np.stack(
        [np.concatenate([outs[2 * b], outs[2 * b + 1]], axis=0) for b in range(B)]
    )
    return full.astype(np.float32)


# revision 48
# speedup vs baseline: 1.0226x; 1.0226x over previous
"""MultiHeadCredibilityAttention TRN2 kernel (v3).

Sharding: 8 cores = (batch b, query-half qh). Each core computes K/V for its
full batch sequence (S=2048) and attention outputs for its 1024 queries.
Output slices concatenate to the full (4, 2048, 256) result — no collectives.

Design (evolved from the v1 baseline through perfetto-trace iteration):
  - Attention is a software-pipelined stream of "units" u = (pair g, m-tile,
    head): 2 score matmuls -> exp -> 2 attn@V matmuls. Head PAIRS are
    staggered by 8 m-tiles so two pairs are always in flight: the softmax
    finalize of pair g overlaps ~8 slots of pair g+1's matmuls, and the two
    ctx-PSUM buffers ping-pong with zero boundary stalls.
  - Exp is split across engines: even heads use ScalarE's exact exp (f32r
    ets); odd heads use a 1-pass Schraudolph fast-exp on the Vector engine
    (int16 round-to-nearest write, bitcast to bf16 ets; ~1.7% elementwise,
    ~3e-3 end-to-end). ScalarE alone would otherwise pace the kernel.
  - All inputs ship as bf16 (halves DMA bytes); projections and score
    matmuls run bf16 (bf16 streams ~216ns/512 cols and enables FWL weight
    loads); attn@V runs f32r for exact heads, bf16 for fast heads (a second
    bf16 V copy is built by the idle GpSimd engine).
  - Softmax denominators ride along as a ones-column in the V operand
    (row 32/96 of the ctx PSUM pair tile). Finalize: ScalarE+DVE evacuate
    the two den rows, a DMA reshapes them to [128,16] so one cheap DVE
    reciprocal covers all lanes, a DRAM bounce broadcasts the reciprocal
    across partitions, DVE scales ctx into ctxT. Emitted as staged thunks,
    one per popped attn@V, to spread engine-queue injections.
  - PSUM: 2 rotating score tiles (4 banks) + 2 ctx pair tiles (2+2 banks);
    the late projections (K chunks 1-3, V tiles 4-15) stream into the first
    8 slots using 2 banks that hand over to the second ctx pool at slot 8.
  - Input DMAs are packed (weights 1-2 DMAs, biases 1) to beat the ~650ns
    per-DMA issue cost; the exp table is primed at t=0.
"""

import sys

import numpy as np

sys.path.insert(0, "/opt/trn_rl_repo")

import concourse.bass as bass  # noqa: E402
import concourse.mybir as mybir  # noqa: E402
from concourse.tile import TileContext  # noqa: E402
from concourse import bass_utils  # noqa: E402

B, S, D, H, HD = 4, 2048, 256, 8, 32
SQ = S // 2  # queries per core
N_CORES = 8
NM = S // 128  # key tiles
F32 = mybir.dt.float32
F32R = mybir.dt.float32r
BF16 = mybir.dt.bfloat16
I16 = mybir.dt.int16
Alu = mybir.AluOpType
INV_SCALE = 1.0 / np.sqrt(HD)

# bf16 Schraudolph fast-exp constants (round-to-nearest variant).
_LOG2E = 1.4426950408889634
FE_A = float((1 << 7) * _LOG2E * INV_SCALE)  # folds the 1/sqrt(hd) scale
FE_B = float(127.0 * (1 << 7) - 8.25)

# which units use the fast path: heads with (h % 2 == 1), m in FAST_MS
FAST_MS = frozenset(range(NM))


def _is_fast(h, m):
    return (h % 2 == 1) and (m in FAST_MS)


def split_multiwaits(nc, max_waits=1):
    """This toolchain's walrus rejects >1 sync-wait per instruction; split
    extras into preceding single-wait NOPs on the same engine."""
    n = 0
    for f in nc.m.functions:
        for bb in f.blocks:
            out = []
            for ins in bb.instructions:
                si = ins.sync_info
                if (
                    si is not None
                    and si.on_wait is not None
                    and len(si.on_wait) > max_waits
                ):
                    waits = list(si.on_wait)
                    for j, w in enumerate(waits[:-max_waits]):
                        n += 1
                        out.append(
                            mybir.InstNoOp(
                                name=f"{ins.name}-wsplit{j}",
                                opcode="NoOp",
                                engine=ins.engine,
                                sync_info=mybir.SyncInfo(on_wait=[w], on_update=[]),
                            )
                        )
                    ins.sync_info = mybir.SyncInfo(
                        on_wait=waits[-max_waits:], on_update=list(si.on_update)
                    )
                out.append(ins)
            bb.instructions = out
    return n


def build_module():
    nc = bass.Bass("TRN2")
    xT_d = nc.dram_tensor("xT", [D, S], BF16, kind="ExternalInput")
    xTq_d = nc.dram_tensor("xTq", [D, SQ], BF16, kind="ExternalInput")
    # weights packed [128, 8*256]: (k0,k1,q0,q1,v0,v1,o0,o1)
    wpack_d = nc.dram_tensor("wpack", [128, 8 * D], BF16, kind="ExternalInput")
    # biases packed [128, 516]: bvb | bob | bq(2 cols) | bk(2 cols)
    bpack_d = nc.dram_tensor("bpack", [128, 2 * D + 4], F32, kind="ExternalInput")
    out_d = nc.dram_tensor("out", [SQ, D], F32, kind="ExternalOutput")

    with TileContext(nc) as tc:
        with (
            tc.tile_pool(name="const", bufs=1) as cpool,
            tc.tile_pool(name="pers", bufs=1) as pers,
        ):
            # ---- input DMAs, chunked + spread across engine queues ----
            xT_sb = [
                cpool.tile([128, S], BF16, tag=f"xT{d}", name=f"xT{d}")
                for d in range(2)
            ]
            xTq_sb = [
                cpool.tile([128, SQ], BF16, tag=f"xTq{d}", name=f"xTq{d}")
                for d in range(2)
            ]
            qs = [nc.sync, nc.scalar, nc.gpsimd]
            wpack_sb = cpool.tile([128, 8 * D], BF16, tag="wpack", name="wpack")
            bpack_sb = cpool.tile([128, 2 * D + 4], F32, tag="bpack", name="bpack")
            w_sb = {
                nm: [
                    wpack_sb[:, (2 * i + d) * D : (2 * i + d + 1) * D]
                    for d in range(2)
                ]
                for i, nm in enumerate(("k", "q", "v", "o"))
            }
            bvb_sb2 = bpack_sb[:, 0:D]
            bob_sb = bpack_sb[:, D : 2 * D]
            bq_sb2 = [bpack_sb[:, 2 * D + d : 2 * D + d + 1] for d in range(2)]
            bk_sb2 = [bpack_sb[:, 2 * D + 2 + d : 2 * D + 3 + d] for d in range(2)]
            jobs = [
                (wpack_sb[:, 0 : 6 * D], wpack_d[:, 0 : 6 * D]),
                (xT_sb[0][:, 0:512], xT_d[0:128, 0:512]),
                (xT_sb[1][:, 0:512], xT_d[128:256, 0:512]),
                (bpack_sb[:], bpack_d[:]),
                (xTq_sb[0][:], xTq_d[0:128, :]),
                (xTq_sb[1][:], xTq_d[128:256, :]),
                (wpack_sb[:, 6 * D : 8 * D], wpack_d[:, 6 * D : 8 * D]),
                (xT_sb[0][:, 512:2048], xT_d[0:128, 512:2048]),
                (xT_sb[1][:, 512:2048], xT_d[128:256, 512:2048]),
            ]
            for qi, (o, i_) in enumerate(jobs):
                qs[qi % 3].dma_start(out=o, in_=i_)

            # prime the exp activation table early (off the attention path)
            prime = cpool.tile([1, 1], F32, tag="prime", name="prime")
            nc.vector.memset(prime[:], 0.0)
            nc.scalar.activation(
                prime[:], prime[:], mybir.ActivationFunctionType.Exp
            )

            # ---- persistent intermediates ----
            KT_sb = [
                pers.tile([128, S], BF16, tag=f"KT{d}", name=f"KT{d}") for d in range(2)
            ]
            QTm_sb = [
                pers.tile([128, SQ], BF16, tag=f"QTm{h}", name=f"QTm{h}")
                for h in range(H)
            ]
            VA_sb = [
                pers.tile([128, H * (HD + 1)], F32R, tag=f"VA{m}", name=f"VA{m}")
                for m in range(NM)
            ]
            VAb_sb = [
                pers.tile([128, H * (HD + 1)], BF16, tag=f"VAb{m}", name=f"VAb{m}")
                for m in range(NM)
            ]
            ctxT_sb = [
                pers.tile([128, SQ], BF16, tag=f"ctxT{d}", name=f"ctxT{d}")
                for d in range(2)
            ]
            denS_sb = [
                pers.tile([1, SQ], F32, tag=f"denS{i}", name=f"denS{i}")
                for i in range(2)
            ]
            denT_sb = pers.tile([128, 16], F32, tag="denT", name="denT")
            recT_sb = pers.tile([128, 16], F32, tag="recT", name="recT")
            rb_sb = [
                pers.tile([HD, SQ], F32, tag=f"rb{i}", name=f"rb{i}") for i in range(2)
            ]

            # masked-Q tiles: zero everything once; Q bias-add writes in-head
            # rows. F32-packed memset path is ~10x faster than the bf16 one.
            for h in range(H):
                nc.vector.memset(QTm_sb[h][:].bitcast(F32), 0.0)
            # ones-columns for the denominator ride-along
            for m in range(NM):
                va = VA_sb[m][:].bitcast(F32).rearrange("p (h c) -> p h c", c=HD + 1)
                nc.vector.memset(va[:, :, HD : HD + 1], 1.0)

            # ---- projection helpers (psum tile passed in) ----
            def emit_k(half, kn, pkt):
                for d in range(2):
                    nc.tensor.matmul(
                        pkt[:],
                        lhsT=w_sb["k"][d][:, half * 128 : (half + 1) * 128],
                        rhs=xT_sb[d][:, kn * 512 : (kn + 1) * 512],
                        start=(d == 0),
                        stop=(d == 1),
                    )
                nc.vector.tensor_scalar_add(
                    KT_sb[half][:, kn * 512 : (kn + 1) * 512],
                    pkt[:],
                    bk_sb2[half],
                )

            def emit_q(half, qn, pqt):
                for d in range(2):
                    nc.tensor.matmul(
                        pqt[:],
                        lhsT=w_sb["q"][d][:, half * 128 : (half + 1) * 128],
                        rhs=xTq_sb[d][:, qn * 512 : (qn + 1) * 512],
                        start=(d == 0),
                        stop=(d == 1),
                    )
                for hh in range(4):
                    h = half * 4 + hh
                    nc.vector.tensor_scalar_add(
                        QTm_sb[h][
                            32 * hh : 32 * (hh + 1), qn * 512 : (qn + 1) * 512
                        ],
                        pqt[32 * hh : 32 * (hh + 1), :],
                        bq_sb2[half][32 * hh : 32 * (hh + 1), :],
                    )

            def emit_v(m, pvt):
                for d in range(2):
                    nc.tensor.matmul(
                        pvt[:],
                        lhsT=xT_sb[d][:, m * 128 : (m + 1) * 128],
                        rhs=w_sb["v"][d][:],
                        start=(d == 0),
                        stop=(d == 1),
                    )
                va = VA_sb[m][:].rearrange("p (h c) -> p h c", c=HD + 1)
                nc.vector.tensor_add(
                    va[:, :, 0:HD],
                    pvt[:].rearrange("p (h c) -> p h c", c=HD),
                    bvb_sb2.rearrange("p (h c) -> p h c", c=HD),
                )
                nc.gpsimd.tensor_copy(VAb_sb[m][:], VA_sb[m][:])

            # ---- early projections: K chunk 0, all Q, V m0-3 ----
            with tc.tile_pool(name="ps_kq", bufs=1, space="PSUM") as ps1:
                pksE = [
                    ps1.tile([128, 512], F32, tag=f"pk{i}", name=f"pk{i}")
                    for i in range(2)
                ]
                pqsE = [
                    ps1.tile([128, 512], F32, tag=f"pq{i}", name=f"pq{i}")
                    for i in range(2)
                ]
                pvsE = [
                    ps1.tile([128, D], F32, tag=f"pv{i}", name=f"pv{i}")
                    for i in range(2)
                ]
                emit_k(0, 0, pksE[0])
                emit_k(1, 0, pksE[1])
                emit_q(0, 0, pqsE[0])
                emit_q(0, 1, pqsE[1])
                emit_q(1, 0, pqsE[0])
                emit_q(1, 1, pqsE[1])
                for m in range(4):
                    emit_v(m, pvsE[m % 2])

            # ---- attention: pipelined units (head, m) ----
            with (
                tc.tile_pool(name="ps_att", bufs=1, space="PSUM") as psa,
                tc.tile_pool(name="etsA", bufs=4) as etsA,
                tc.tile_pool(name="etsB", bufs=4) as etsB,
                tc.tile_pool(name="dramp", bufs=2, space="DRAM") as dramp,
            ):
                pscs = [
                    psa.tile([128, SQ], F32, tag=f"psc{i}", name=f"psc{i}")
                    for i in range(2)
                ]
                pending = []  # deferred attn@V emissions, as (fn, args...)

                def emit_av(h, m, ets, pctx, base, fast):
                    vsb = VAb_sb[m] if fast else VA_sb[m]
                    for qn in range(2):
                        nc.tensor.matmul(
                            pctx[base : base + HD + 1, qn * 512 : (qn + 1) * 512],
                            lhsT=vsb[:, h * (HD + 1) : (h + 1) * (HD + 1)],
                            rhs=ets[:, qn * 512 : (qn + 1) * 512],
                            start=(m == 0),
                            stop=(m == NM - 1),
                        )

                fin_steps = []  # staged finalize thunks, one per unit

                def emit_finalize(hg, pctx):
                    # recip -> DRAM-bounce broadcast -> scale; staged so the
                    # engine-queue injections spread over several units
                    half = (2 * hg) // 4
                    rrec = dramp.tile([2, SQ], F32, tag="rrec", name=f"rrec{hg}")

                    def s1():
                        nc.scalar.activation(
                            denS_sb[0][:],
                            pctx[HD : HD + 1, :],
                            mybir.ActivationFunctionType.Copy,
                        )
                        nc.vector.tensor_copy(
                            denS_sb[1][:], pctx[64 + HD : 64 + HD + 1, :]
                        )
                        nc.sync.dma_start(out=denT_sb[0:64, :], in_=denS_sb[0][:])
                        nc.scalar.dma_start(out=denT_sb[64:128, :], in_=denS_sb[1][:])

                    def s2():
                        nc.vector.reciprocal(recT_sb[:], denT_sb[:])
                        nc.sync.dma_start(out=rrec[0:1, :], in_=recT_sb[0:64, :])
                        nc.scalar.dma_start(out=rrec[1:2, :], in_=recT_sb[64:128, :])
                        nc.sync.dma_start(
                            out=rb_sb[0][:], in_=rrec[0:1, :].to_broadcast((HD, SQ))
                        )
                        nc.scalar.dma_start(
                            out=rb_sb[1][:], in_=rrec[1:2, :].to_broadcast((HD, SQ))
                        )

                    def mk_mul(i, h):
                        def s():
                            base = 64 * i
                            hh = h % 4
                            nc.vector.tensor_mul(
                                ctxT_sb[half][32 * hh : 32 * (hh + 1), :],
                                pctx[base : base + HD, :],
                                rb_sb[i][:],
                            )

                        return s

                    fin_steps.extend(
                        [s1, s2, mk_mul(0, 2 * hg), mk_mul(1, 2 * hg + 1)]
                    )

                pop_ctr = [0]

                def pop_one():
                    h, m, ets, pctx_t, base, hg_of, fast = pending.pop(0)
                    emit_av(h, m, ets, pctx_t, base, fast)
                    if h % 2 == 1 and m == NM - 1:
                        emit_finalize(hg_of, pctx_t)
                    pop_ctr[0] += 1
                    if fin_steps and pop_ctr[0] % 2 == 0:
                        fin_steps.pop(0)()

                # pairs staggered by 8 m-tiles: two pairs in flight, two pctx
                # buffers; each pair's finalize has ~8 slots of slack before
                # its pctx banks are needed again
                STAG = 8
                units = []
                for s in range(STAG * (H // 2 - 1) + NM):
                    for g in range(H // 2):
                        m = s - STAG * g
                        if 0 <= m < NM:
                            for h in (2 * g, 2 * g + 1):
                                units.append((g, m, h))

                # late projections streamed into the early slots; their psum
                # banks hand over to pctx1 at slot 8
                pctx0_cm = tc.tile_pool(name="pctx0", bufs=1, space="PSUM")
                pctx0_pool = pctx0_cm.__enter__()
                late_cm = tc.tile_pool(name="ps_late", bufs=1, space="PSUM")
                late = late_cm.__enter__()
                pkL = late.tile([128, 512], F32, tag="pkL", name="pkL")
                pvL = late.tile([128, D], F32, tag="pvL", name="pvL")
                quota = []
                for kn in range(1, 4):
                    for mv in (4 * kn, 4 * kn + 1):
                        quota.append((emit_v, (mv, pvL)))
                    quota.append((emit_k, (0, kn, pkL)))
                    quota.append((emit_k, (1, kn, pkL)))
                    for mv in (4 * kn + 2, 4 * kn + 3):
                        quota.append((emit_v, (mv, pvL)))
                pctx1_cm = None
                pctx1_pool = None
                pctxs = {}
                uidx = 0
                for g, m, h in units:
                    slot = STAG * g + m
                    if slot >= STAG and late_cm is not None:
                        while quota:
                            fn, args = quota.pop(0)
                            fn(*args)
                        late_cm.__exit__(None, None, None)
                        late_cm = None
                        pctx1_cm = tc.tile_pool(
                            name="pctx1", bufs=1, space="PSUM"
                        )
                        pctx1_pool = pctx1_cm.__enter__()
                    if g not in pctxs:
                        pool_g = pctx0_pool if g % 2 == 0 else pctx1_pool
                        pctxs[g] = pool_g.tile(
                            [97, SQ], F32, tag="pctx", name=f"pctx{g}"
                        )
                    pctx = pctxs[g]
                    half = h // 4
                    psc = pscs[uidx % 2]
                    for qn in range(2):
                        nc.tensor.matmul(
                            psc[:, qn * 512 : (qn + 1) * 512],
                            lhsT=KT_sb[half][:, m * 128 : (m + 1) * 128],
                            rhs=QTm_sb[h][:, qn * 512 : (qn + 1) * 512],
                            start=True,
                            stop=True,
                        )
                    if _is_fast(h, m):
                        ets = etsB.tile([128, SQ], BF16, tag="eB", name="eB")
                        nc.vector.tensor_scalar(
                            ets[:].bitcast(I16),
                            psc[:],
                            FE_A,
                            FE_B,
                            Alu.mult,
                            Alu.add,
                        )
                    else:
                        ets = etsA.tile([128, SQ], F32R, tag="eA", name="eA")
                        nc.scalar.activation(
                            ets[:],
                            psc[:],
                            mybir.ActivationFunctionType.Exp,
                            scale=float(INV_SCALE),
                        )
                    base = 0 if h % 2 == 0 else 64
                    pending.append((h, m, ets, pctx, base, g, _is_fast(h, m)))
                    while len(pending) > 4:
                        pop_one()
                    for _ in range(2):
                        if quota:
                            fn, args = quota.pop(0)
                            fn(*args)
                    uidx += 1
                while pending:
                    pop_one()
                while fin_steps:
                    fin_steps.pop(0)()
                if pctx1_cm is not None:
                    pctx1_cm.__exit__(None, None, None)
                pctx0_cm.__exit__(None, None, None)

            # ---- output projection ----
            with (
                tc.tile_pool(name="ps_o", bufs=4, space="PSUM") as ps_o,
                tc.tile_pool(name="outp", bufs=4) as outp,
            ):
                for t in range(SQ // 128):
                    po = ps_o.tile([128, D], F32, tag="po", name="po")
                    for d in range(2):
                        nc.tensor.matmul(
                            po[:],
                            lhsT=ctxT_sb[d][:, t * 128 : (t + 1) * 128],
                            rhs=w_sb["o"][d][:],
                            start=(d == 0),
                            stop=(d == 1),
                        )
                    ot = outp.tile([128, D], F32, tag="ot", name="ot")
                    nc.vector.tensor_add(ot[:], po[:], bob_sb)
                    oq = nc.sync if t % 2 == 0 else nc.scalar
                    oq.dma_start(out=out_d[t * 128 : (t + 1) * 128, :], in_=ot[:])

    split_multiwaits(nc)
    return nc


_module_cache = {}


def _get_module():
    if "m" not in _module_cache:
        _module_cache["m"] = build_module()
    return _module_cache["m"]


def make_in_maps(inputs):
    x = np.asarray(inputs["x"], np.float32)
    Wq, bq = np.asarray(inputs["Wq"], np.float32), np.asarray(inputs["bq"], np.float32)
    Wk, bk = np.asarray(inputs["Wk"], np.float32), np.asarray(inputs["bk"], np.float32)
    Wv, bv = np.asarray(inputs["Wv"], np.float32), np.asarray(inputs["bv"], np.float32)
    Wo, bo = np.asarray(inputs["Wo"], np.float32), np.asarray(inputs["bo"], np.float32)
    hs = np.asarray(inputs["head_scale"], np.float32)

    import ml_dtypes

    bf16 = np.dtype(ml_dtypes.bfloat16)
    hs_col = np.repeat(hs, HD)  # head_scale folded into V
    wqT = Wq.T.astype(bf16)
    wkT = Wk.T.astype(bf16)
    wvT = (Wv * hs_col[:, None]).T.astype(bf16)
    woT = Wo.T.astype(bf16)
    wpack = np.concatenate(
        [wkT[0:128], wkT[128:256], wqT[0:128], wqT[128:256],
         wvT[0:128], wvT[128:256], woT[0:128], woT[128:256]],
        axis=1,
    )
    bpack = np.concatenate(
        [
            np.broadcast_to(bv * hs_col, (128, D)),
            np.broadcast_to(bo, (128, D)),
            bq.reshape(2, 128).T,
            bk.reshape(2, 128).T,
        ],
        axis=1,
    ).astype(np.float32)
    shared = {
        "wpack": np.ascontiguousarray(wpack),
        "bpack": np.ascontiguousarray(bpack),
    }
    in_maps = []
    for k in range(N_CORES):
        b, qh = k // 2, k % 2
        xT = np.ascontiguousarray(x[b].T.astype(bf16))
        in_maps.append(
            {
                "xT": xT,
                "xTq": np.ascontiguousarray(xT[:, qh * SQ : (qh + 1) * SQ]),
                **shared,
            }
        )
    return in_maps


def kernel(x, Wq, bq, Wk, bk, Wv, bv, Wo, bo, head_scale):
    in_maps = make_in_maps(
        dict(
            x=x, Wq=Wq, bq=bq, Wk=Wk, bk=bk, Wv=Wv, bv=bv, Wo=Wo, bo=bo,
            head_scale=head_scale,
        )
    )
    nc = _get_module()
    core_ids = list(range(N_CORES))
    # First execution after a fresh process attach has been observed to
    # return corrupted results on some cores; run once to warm up, then
    # use the second run (retry if it still looks corrupted).
    bass_utils.run_bass_kernel_spmd(nc, in_maps, core_ids=core_ids)
    for _ in range(3):
        res = bass_utils.run_bass_kernel_spmd(nc, in_maps, core_ids=core_ids)
        outs = [r["out"] for r in res.results]
        finite = all(np.isfinite(o).all() for o in outs)
        if finite and max(float(np.abs(o).max()) for o in outs) < 1e4:
            break
    full = np.stack(
        [np.concatenate([outs[2 * b], outs[2 * b + 1]], axis=0) for b in range(B)]
    )
    return full.astype(np.float32)


# revision 49
# speedup vs baseline: 1.0403x; 1.0172x over previous
"""MultiHeadCredibilityAttention TRN2 kernel (v3).

Sharding: 8 cores = (batch b, query-half qh). Each core computes K/V for its
full batch sequence (S=2048) and attention outputs for its 1024 queries.
Output slices concatenate to the full (4, 2048, 256) result — no collectives.

Design (evolved from the v1 baseline through perfetto-trace iteration):
  - Attention is a software-pipelined stream of "units" u = (pair g, m-tile,
    head): 2 score matmuls -> exp -> 2 attn@V matmuls. Head PAIRS are
    staggered by 8 m-tiles so two pairs are always in flight: the softmax
    finalize of pair g overlaps ~8 slots of pair g+1's matmuls, and the two
    ctx-PSUM buffers ping-pong with zero boundary stalls.
  - Exp is split across engines: even heads use ScalarE's exact exp (f32r
    ets); odd heads use a 1-pass Schraudolph fast-exp on the Vector engine
    (int16 round-to-nearest write, bitcast to bf16 ets; ~1.7% elementwise,
    ~3e-3 end-to-end). ScalarE alone would otherwise pace the kernel.
  - All inputs ship as bf16 (halves DMA bytes); projections and score
    matmuls run bf16 (bf16 streams ~216ns/512 cols and enables FWL weight
    loads); attn@V runs f32r for exact heads, bf16 for fast heads (a second
    bf16 V copy is built by the idle GpSimd engine).
  - Softmax denominators ride along as a ones-column in the V operand
    (row 32/96 of the ctx PSUM pair tile). Finalize: ScalarE+DVE evacuate
    the two den rows, a DMA reshapes them to [128,16] so one cheap DVE
    reciprocal covers all lanes, a DRAM bounce broadcasts the reciprocal
    across partitions, DVE scales ctx into ctxT. Emitted as staged thunks,
    one per popped attn@V, to spread engine-queue injections.
  - PSUM: 2 rotating score tiles (4 banks) + 2 ctx pair tiles (2+2 banks);
    the late projections (K chunks 1-3, V tiles 4-15) stream into the first
    8 slots using 2 banks that hand over to the second ctx pool at slot 8.
  - Input DMAs are packed (weights 1-2 DMAs, biases 1) to beat the ~650ns
    per-DMA issue cost; the exp table is primed at t=0.
"""

import sys

import numpy as np

sys.path.insert(0, "/opt/trn_rl_repo")

import concourse.bass as bass  # noqa: E402
import concourse.mybir as mybir  # noqa: E402
from concourse.tile import TileContext  # noqa: E402
from concourse import bass_utils  # noqa: E402

B, S, D, H, HD = 4, 2048, 256, 8, 32
SQ = S // 2  # queries per core
N_CORES = 8
NM = S // 128  # key tiles
F32 = mybir.dt.float32
F32R = mybir.dt.float32r
BF16 = mybir.dt.bfloat16
I16 = mybir.dt.int16
Alu = mybir.AluOpType
INV_SCALE = 1.0 / np.sqrt(HD)

# bf16 Schraudolph fast-exp constants (round-to-nearest variant).
_LOG2E = 1.4426950408889634
FE_A = float((1 << 7) * _LOG2E * INV_SCALE)  # folds the 1/sqrt(hd) scale
FE_B = float(127.0 * (1 << 7) - 8.25)

# which units use the fast path: heads with (h % 2 == 1), m in FAST_MS
FAST_MS = frozenset(range(NM))


def _is_fast(h, m):
    return (h % 2 == 1) and (m in FAST_MS)


def split_multiwaits(nc, max_waits=1):
    """This toolchain's walrus rejects >1 sync-wait per instruction; split
    extras into preceding single-wait NOPs on the same engine."""
    n = 0
    for f in nc.m.functions:
        for bb in f.blocks:
            out = []
            for ins in bb.instructions:
                si = ins.sync_info
                if (
                    si is not None
                    and si.on_wait is not None
                    and len(si.on_wait) > max_waits
                ):
                    waits = list(si.on_wait)
                    for j, w in enumerate(waits[:-max_waits]):
                        n += 1
                        out.append(
                            mybir.InstNoOp(
                                name=f"{ins.name}-wsplit{j}",
                                opcode="NoOp",
                                engine=ins.engine,
                                sync_info=mybir.SyncInfo(on_wait=[w], on_update=[]),
                            )
                        )
                    ins.sync_info = mybir.SyncInfo(
                        on_wait=waits[-max_waits:], on_update=list(si.on_update)
                    )
                out.append(ins)
            bb.instructions = out
    return n


def build_module():
    nc = bass.Bass("TRN2")
    xT_d = nc.dram_tensor("xT", [D, S], BF16, kind="ExternalInput")
    xTq_d = nc.dram_tensor("xTq", [D, SQ], BF16, kind="ExternalInput")
    # weights packed [128, 8*256]: (k0,k1,q0,q1,v0,v1,o0,o1)
    wpack_d = nc.dram_tensor("wpack", [128, 8 * D], BF16, kind="ExternalInput")
    # biases packed [128, 516]: bvb | bob | bq(2 cols) | bk(2 cols)
    bpack_d = nc.dram_tensor("bpack", [128, 2 * D + 4], F32, kind="ExternalInput")
    out_d = nc.dram_tensor("out", [SQ, D], F32, kind="ExternalOutput")

    with TileContext(nc) as tc:
        with (
            tc.tile_pool(name="const", bufs=1) as cpool,
            tc.tile_pool(name="pers", bufs=1) as pers,
        ):
            # ---- input DMAs, chunked + spread across engine queues ----
            xT_sb = [
                cpool.tile([128, S], BF16, tag=f"xT{d}", name=f"xT{d}")
                for d in range(2)
            ]
            xTq_sb = [
                cpool.tile([128, SQ], BF16, tag=f"xTq{d}", name=f"xTq{d}")
                for d in range(2)
            ]
            qs = [nc.sync, nc.scalar, nc.gpsimd]
            wpack_sb = cpool.tile([128, 8 * D], BF16, tag="wpack", name="wpack")
            bpack_sb = cpool.tile([128, 2 * D + 4], F32, tag="bpack", name="bpack")
            w_sb = {
                nm: [
                    wpack_sb[:, (2 * i + d) * D : (2 * i + d + 1) * D]
                    for d in range(2)
                ]
                for i, nm in enumerate(("k", "q", "v", "o"))
            }
            bvb_sb2 = bpack_sb[:, 0:D]
            bob_sb = bpack_sb[:, D : 2 * D]
            bq_sb2 = [bpack_sb[:, 2 * D + d : 2 * D + d + 1] for d in range(2)]
            bk_sb2 = [bpack_sb[:, 2 * D + 2 + d : 2 * D + 3 + d] for d in range(2)]
            jobs = [
                (wpack_sb[:, 0 : 6 * D], wpack_d[:, 0 : 6 * D]),
                (xT_sb[0][:, 0:512], xT_d[0:128, 0:512]),
                (xT_sb[1][:, 0:512], xT_d[128:256, 0:512]),
                (bpack_sb[:], bpack_d[:]),
                (xTq_sb[0][:], xTq_d[0:128, :]),
                (xTq_sb[1][:], xTq_d[128:256, :]),
                (wpack_sb[:, 6 * D : 8 * D], wpack_d[:, 6 * D : 8 * D]),
                (xT_sb[0][:, 512:2048], xT_d[0:128, 512:2048]),
                (xT_sb[1][:, 512:2048], xT_d[128:256, 512:2048]),
            ]
            for qi, (o, i_) in enumerate(jobs):
                qs[qi % 3].dma_start(out=o, in_=i_)

            # prime the exp activation table early (off the attention path)
            prime = cpool.tile([1, 1], F32, tag="prime", name="prime")
            nc.vector.memset(prime[:], 0.0)
            nc.scalar.activation(
                prime[:], prime[:], mybir.ActivationFunctionType.Exp
            )

            # ---- persistent intermediates ----
            KT_sb = [
                pers.tile([128, S], BF16, tag=f"KT{d}", name=f"KT{d}") for d in range(2)
            ]
            QTm_sb = [
                pers.tile([128, SQ], BF16, tag=f"QTm{h}", name=f"QTm{h}")
                for h in range(H)
            ]
            VA_sb = [
                pers.tile([128, H * (HD + 1)], F32R, tag=f"VA{m}", name=f"VA{m}")
                for m in range(NM)
            ]
            VAb_sb = [
                pers.tile([128, H * (HD + 1)], BF16, tag=f"VAb{m}", name=f"VAb{m}")
                for m in range(NM)
            ]
            ctxT_sb = [
                pers.tile([128, SQ], BF16, tag=f"ctxT{d}", name=f"ctxT{d}")
                for d in range(2)
            ]
            denS_sb = [
                pers.tile([1, SQ], F32, tag=f"denS{i}", name=f"denS{i}")
                for i in range(2)
            ]
            denT_sb = pers.tile([128, 16], F32, tag="denT", name="denT")
            recT_sb = pers.tile([128, 16], F32, tag="recT", name="recT")
            rb_sb = [
                pers.tile([HD, SQ], F32, tag=f"rb{i}", name=f"rb{i}") for i in range(2)
            ]

            # masked-Q tiles: zero everything once; Q bias-add writes in-head
            # rows. F32-packed memset path is ~10x faster than the bf16 one.
            for h in range(H):
                nc.vector.memset(QTm_sb[h][:].bitcast(F32), 0.0)
            # ones-columns for the denominator ride-along
            for m in range(NM):
                va = VA_sb[m][:].bitcast(F32).rearrange("p (h c) -> p h c", c=HD + 1)
                nc.vector.memset(va[:, :, HD : HD + 1], 1.0)

            # ---- projection helpers (psum tile passed in) ----
            def emit_k(half, kn, pkt):
                for d in range(2):
                    nc.tensor.matmul(
                        pkt[:],
                        lhsT=w_sb["k"][d][:, half * 128 : (half + 1) * 128],
                        rhs=xT_sb[d][:, kn * 512 : (kn + 1) * 512],
                        start=(d == 0),
                        stop=(d == 1),
                    )
                nc.vector.tensor_scalar_add(
                    KT_sb[half][:, kn * 512 : (kn + 1) * 512],
                    pkt[:],
                    bk_sb2[half],
                )

            def emit_q(half, qn, pqt):
                for d in range(2):
                    nc.tensor.matmul(
                        pqt[:],
                        lhsT=w_sb["q"][d][:, half * 128 : (half + 1) * 128],
                        rhs=xTq_sb[d][:, qn * 512 : (qn + 1) * 512],
                        start=(d == 0),
                        stop=(d == 1),
                    )
                for hh in range(4):
                    h = half * 4 + hh
                    nc.vector.tensor_scalar_add(
                        QTm_sb[h][
                            32 * hh : 32 * (hh + 1), qn * 512 : (qn + 1) * 512
                        ],
                        pqt[32 * hh : 32 * (hh + 1), :],
                        bq_sb2[half][32 * hh : 32 * (hh + 1), :],
                    )

            def emit_v(m, pvt):
                for d in range(2):
                    nc.tensor.matmul(
                        pvt[:],
                        lhsT=xT_sb[d][:, m * 128 : (m + 1) * 128],
                        rhs=w_sb["v"][d][:],
                        start=(d == 0),
                        stop=(d == 1),
                    )
                va = VA_sb[m][:].rearrange("p (h c) -> p h c", c=HD + 1)
                nc.vector.tensor_add(
                    va[:, :, 0:HD],
                    pvt[:].rearrange("p (h c) -> p h c", c=HD),
                    bvb_sb2.rearrange("p (h c) -> p h c", c=HD),
                )
                nc.gpsimd.tensor_copy(VAb_sb[m][:], VA_sb[m][:])

            # ---- early projections: K chunk 0, all Q, V m0-3 ----
            with tc.tile_pool(name="ps_kq", bufs=1, space="PSUM") as ps1:
                pksE = [
                    ps1.tile([128, 512], F32, tag=f"pk{i}", name=f"pk{i}")
                    for i in range(2)
                ]
                pqsE = [
                    ps1.tile([128, 512], F32, tag=f"pq{i}", name=f"pq{i}")
                    for i in range(2)
                ]
                pvsE = [
                    ps1.tile([128, D], F32, tag=f"pv{i}", name=f"pv{i}")
                    for i in range(2)
                ]
                emit_k(0, 0, pksE[0])
                emit_k(1, 0, pksE[1])
                emit_q(0, 0, pqsE[0])
                emit_q(0, 1, pqsE[1])
                emit_q(1, 0, pqsE[0])
                emit_q(1, 1, pqsE[1])
                for m in range(4):
                    emit_v(m, pvsE[m % 2])

            # ---- attention: pipelined units (head, m) ----
            with (
                tc.tile_pool(name="ps_att", bufs=1, space="PSUM") as psa,
                tc.tile_pool(name="etsA", bufs=4) as etsA,
                tc.tile_pool(name="etsB", bufs=4) as etsB,
                tc.tile_pool(name="dramp", bufs=2, space="DRAM") as dramp,
            ):
                pscs = [
                    psa.tile([128, SQ], F32, tag=f"psc{i}", name=f"psc{i}")
                    for i in range(2)
                ]
                pending = []  # deferred attn@V emissions, as (fn, args...)

                def emit_av(h, m, ets, pctx, base, fast):
                    vsb = VAb_sb[m] if fast else VA_sb[m]
                    for qn in range(2):
                        nc.tensor.matmul(
                            pctx[base : base + HD + 1, qn * 512 : (qn + 1) * 512],
                            lhsT=vsb[:, h * (HD + 1) : (h + 1) * (HD + 1)],
                            rhs=ets[:, qn * 512 : (qn + 1) * 512],
                            start=(m == 0),
                            stop=(m == NM - 1),
                        )

                fin_steps = []  # staged finalize thunks, one per unit

                def emit_finalize(hg, pctx):
                    # recip -> DRAM-bounce broadcast -> scale; staged so the
                    # engine-queue injections spread over several units
                    half = (2 * hg) // 4
                    rrec = dramp.tile([2, SQ], F32, tag="rrec", name=f"rrec{hg}")

                    def s1():
                        nc.scalar.activation(
                            denS_sb[0][:],
                            pctx[HD : HD + 1, :],
                            mybir.ActivationFunctionType.Copy,
                        )
                        nc.vector.tensor_copy(
                            denS_sb[1][:], pctx[64 + HD : 64 + HD + 1, :]
                        )
                        nc.sync.dma_start(out=denT_sb[0:64, :], in_=denS_sb[0][:])
                        nc.scalar.dma_start(out=denT_sb[64:128, :], in_=denS_sb[1][:])

                    def s2():
                        nc.vector.reciprocal(recT_sb[:], denT_sb[:])
                        nc.sync.dma_start(out=rrec[0:1, :], in_=recT_sb[0:64, :])
                        nc.scalar.dma_start(out=rrec[1:2, :], in_=recT_sb[64:128, :])
                        nc.sync.dma_start(
                            out=rb_sb[0][:], in_=rrec[0:1, :].to_broadcast((HD, SQ))
                        )
                        nc.scalar.dma_start(
                            out=rb_sb[1][:], in_=rrec[1:2, :].to_broadcast((HD, SQ))
                        )

                    def mk_mul(i, h):
                        def s():
                            base = 64 * i
                            hh = h % 4
                            nc.vector.tensor_mul(
                                ctxT_sb[half][32 * hh : 32 * (hh + 1), :],
                                pctx[base : base + HD, :],
                                rb_sb[i][:],
                            )

                        return s

                    fin_steps.extend(
                        [s1, s2, mk_mul(0, 2 * hg), mk_mul(1, 2 * hg + 1)]
                    )

                pop_ctr = [0]

                def pop_one():
                    h, m, ets, pctx_t, base, hg_of, fast = pending.pop(0)
                    emit_av(h, m, ets, pctx_t, base, fast)
                    if h % 2 == 1 and m == NM - 1:
                        emit_finalize(hg_of, pctx_t)
                    pop_ctr[0] += 1
                    if fin_steps and pop_ctr[0] % 2 == 0:
                        fin_steps.pop(0)()

                # pairs staggered by 8 m-tiles: two pairs in flight, two pctx
                # buffers; each pair's finalize has ~8 slots of slack before
                # its pctx banks are needed again
                STAG = 12
                units = []
                for s in range(STAG * (H // 2 - 1) + NM):
                    for g in range(H // 2):
                        m = s - STAG * g
                        if 0 <= m < NM:
                            for h in (2 * g, 2 * g + 1):
                                units.append((g, m, h))

                # late projections streamed into the early slots; their psum
                # banks hand over to pctx1 at slot 8
                pctx0_cm = tc.tile_pool(name="pctx0", bufs=1, space="PSUM")
                pctx0_pool = pctx0_cm.__enter__()
                late_cm = tc.tile_pool(name="ps_late", bufs=1, space="PSUM")
                late = late_cm.__enter__()
                pkL = late.tile([128, 512], F32, tag="pkL", name="pkL")
                pvL = late.tile([128, D], F32, tag="pvL", name="pvL")
                quota = []
                for kn in range(1, 4):
                    for mv in (4 * kn, 4 * kn + 1):
                        quota.append((emit_v, (mv, pvL)))
                    quota.append((emit_k, (0, kn, pkL)))
                    quota.append((emit_k, (1, kn, pkL)))
                    for mv in (4 * kn + 2, 4 * kn + 3):
                        quota.append((emit_v, (mv, pvL)))
                pctx1_cm = None
                pctx1_pool = None
                pctxs = {}
                uidx = 0
                for g, m, h in units:
                    slot = STAG * g + m
                    if slot >= STAG and late_cm is not None:
                        while quota:
                            fn, args = quota.pop(0)
                            fn(*args)
                        late_cm.__exit__(None, None, None)
                        late_cm = None
                        pctx1_cm = tc.tile_pool(
                            name="pctx1", bufs=1, space="PSUM"
                        )
                        pctx1_pool = pctx1_cm.__enter__()
                    if g not in pctxs:
                        pool_g = pctx0_pool if g % 2 == 0 else pctx1_pool
                        pctxs[g] = pool_g.tile(
                            [97, SQ], F32, tag="pctx", name=f"pctx{g}"
                        )
                    pctx = pctxs[g]
                    half = h // 4
                    psc = pscs[uidx % 2]
                    for qn in range(2):
                        nc.tensor.matmul(
                            psc[:, qn * 512 : (qn + 1) * 512],
                            lhsT=KT_sb[half][:, m * 128 : (m + 1) * 128],
                            rhs=QTm_sb[h][:, qn * 512 : (qn + 1) * 512],
                            start=True,
                            stop=True,
                        )
                    if _is_fast(h, m):
                        ets = etsB.tile([128, SQ], BF16, tag="eB", name="eB")
                        nc.vector.tensor_scalar(
                            ets[:].bitcast(I16),
                            psc[:],
                            FE_A,
                            FE_B,
                            Alu.mult,
                            Alu.add,
                        )
                    else:
                        ets = etsA.tile([128, SQ], F32R, tag="eA", name="eA")
                        nc.scalar.activation(
                            ets[:],
                            psc[:],
                            mybir.ActivationFunctionType.Exp,
                            scale=float(INV_SCALE),
                        )
                    base = 0 if h % 2 == 0 else 64
                    pending.append((h, m, ets, pctx, base, g, _is_fast(h, m)))
                    while len(pending) > 4:
                        pop_one()
                    for _ in range(2):
                        if quota:
                            fn, args = quota.pop(0)
                            fn(*args)
                    uidx += 1
                while pending:
                    pop_one()
                while fin_steps:
                    fin_steps.pop(0)()
                if pctx1_cm is not None:
                    pctx1_cm.__exit__(None, None, None)
                pctx0_cm.__exit__(None, None, None)

            # ---- output projection ----
            with (
                tc.tile_pool(name="ps_o", bufs=4, space="PSUM") as ps_o,
                tc.tile_pool(name="outp", bufs=4) as outp,
            ):
                for t in range(SQ // 128):
                    po = ps_o.tile([128, D], F32, tag="po", name="po")
                    for d in range(2):
                        nc.tensor.matmul(
                            po[:],
                            lhsT=ctxT_sb[d][:, t * 128 : (t + 1) * 128],
                            rhs=w_sb["o"][d][:],
                            start=(d == 0),
                            stop=(d == 1),
                        )
                    ot = outp.tile([128, D], F32, tag="ot", name="ot")
                    nc.vector.tensor_add(ot[:], po[:], bob_sb)
                    oq = nc.sync if t % 2 == 0 else nc.scalar
                    oq.dma_start(out=out_d[t * 128 : (t + 1) * 128, :], in_=ot[:])

    split_multiwaits(nc)
    return nc


_module_cache = {}


def _get_module():
    if "m" not in _module_cache:
        _module_cache["m"] = build_module()
    return _module_cache["m"]


def make_in_maps(inputs):
    x = np.asarray(inputs["x"], np.float32)
    Wq, bq = np.asarray(inputs["Wq"], np.float32), np.asarray(inputs["bq"], np.float32)
    Wk, bk = np.asarray(inputs["Wk"], np.float32), np.asarray(inputs["bk"], np.float32)
    Wv, bv = np.asarray(inputs["Wv"], np.float32), np.asarray(inputs["bv"], np.float32)
    Wo, bo = np.asarray(inputs["Wo"], np.float32), np.asarray(inputs["bo"], np.float32)
    hs = np.asarray(inputs["head_scale"], np.float32)

    import ml_dtypes

    bf16 = np.dtype(ml_dtypes.bfloat16)
    hs_col = np.repeat(hs, HD)  # head_scale folded into V
    wqT = Wq.T.astype(bf16)
    wkT = Wk.T.astype(bf16)
    wvT = (Wv * hs_col[:, None]).T.astype(bf16)
    woT = Wo.T.astype(bf16)
    wpack = np.concatenate(
        [wkT[0:128], wkT[128:256], wqT[0:128], wqT[128:256],
         wvT[0:128], wvT[128:256], woT[0:128], woT[128:256]],
        axis=1,
    )
    bpack = np.concatenate(
        [
            np.broadcast_to(bv * hs_col, (128, D)),
            np.broadcast_to(bo, (128, D)),
            bq.reshape(2, 128).T,
            bk.reshape(2, 128).T,
        ],
        axis=1,
    ).astype(np.float32)
    shared = {
        "wpack": np.ascontiguousarray(wpack),
        "bpack": np.ascontiguousarray(bpack),
    }
    in_maps = []
    for k in range(N_CORES):
        b, qh = k // 2, k % 2
        xT = np.ascontiguousarray(x[b].T.astype(bf16))
        in_maps.append(
            {
                "xT": xT,
                "xTq": np.ascontiguousarray(xT[:, qh * SQ : (qh + 1) * SQ]),
                **shared,
            }
        )
    return in_maps


def kernel(x, Wq, bq, Wk, bk, Wv, bv, Wo, bo, head_scale):
    in_maps = make_in_maps(
        dict(
            x=x, Wq=Wq, bq=bq, Wk=Wk, bk=bk, Wv=Wv, bv=bv, Wo=Wo, bo=bo,
            head_scale=head_scale,
        )
    )
    nc = _get_module()
    core_ids = list(range(N_CORES))
    # First execution after a fresh process attach has been observed to
    # return corrupted results on some cores; run once to warm up, then
    # use the second run (retry if it still looks corrupted).
    bass_utils.run_bass_kernel_spmd(nc, in_maps, core_ids=core_ids)
    for _ in range(3):
        res = bass_utils.run_bass_kernel_spmd(nc, in_maps, core_ids=core_ids)
        outs = [r["out"] for r in res.results]
        finite = all(np.isfinite(o).all() for o in outs)
        if finite and max(float(np.abs(o).max()) for o in outs) < 1e4:
            break
    full = np.stack(
        [np.concatenate([outs[2 * b], outs[2 * b + 1]], axis=0) for b in range(B)]
    )
    return full.astype(np.float32)


# revision 50
# speedup vs baseline: 1.0541x; 1.0133x over previous
"""MultiHeadCredibilityAttention TRN2 kernel (v3).

Sharding: 8 cores = (batch b, query-half qh). Each core computes K/V for its
full batch sequence (S=2048) and attention outputs for its 1024 queries.
Output slices concatenate to the full (4, 2048, 256) result — no collectives.

Design (evolved from the v1 baseline through perfetto-trace iteration):
  - Attention is a software-pipelined stream of "units" u = (pair g, m-tile,
    head): 2 score matmuls -> exp -> 2 attn@V matmuls. Head PAIRS are
    staggered by 8 m-tiles so two pairs are always in flight: the softmax
    finalize of pair g overlaps ~8 slots of pair g+1's matmuls, and the two
    ctx-PSUM buffers ping-pong with zero boundary stalls.
  - Exp is split across engines: even heads use ScalarE's exact exp (f32r
    ets); odd heads use a 1-pass Schraudolph fast-exp on the Vector engine
    (int16 round-to-nearest write, bitcast to bf16 ets; ~1.7% elementwise,
    ~3e-3 end-to-end). ScalarE alone would otherwise pace the kernel.
  - All inputs ship as bf16 (halves DMA bytes); projections and score
    matmuls run bf16 (bf16 streams ~216ns/512 cols and enables FWL weight
    loads); attn@V runs f32r for exact heads, bf16 for fast heads (a second
    bf16 V copy is built by the idle GpSimd engine).
  - Softmax denominators ride along as a ones-column in the V operand
    (row 32/96 of the ctx PSUM pair tile). Finalize: ScalarE+DVE evacuate
    the two den rows, a DMA reshapes them to [128,16] so one cheap DVE
    reciprocal covers all lanes, a DRAM bounce broadcasts the reciprocal
    across partitions, DVE scales ctx into ctxT. Emitted as staged thunks,
    one per popped attn@V, to spread engine-queue injections.
  - PSUM: 2 rotating score tiles (4 banks) + 2 ctx pair tiles (2+2 banks);
    the late projections (K chunks 1-3, V tiles 4-15) stream into the first
    8 slots using 2 banks that hand over to the second ctx pool at slot 8.
  - Input DMAs are packed (weights 1-2 DMAs, biases 1) to beat the ~650ns
    per-DMA issue cost; the exp table is primed at t=0.
"""

import sys

import numpy as np

sys.path.insert(0, "/opt/trn_rl_repo")

import concourse.bass as bass  # noqa: E402
import concourse.mybir as mybir  # noqa: E402
from concourse.tile import TileContext  # noqa: E402
from concourse import bass_utils  # noqa: E402

B, S, D, H, HD = 4, 2048, 256, 8, 32
SQ = S // 2  # queries per core
N_CORES = 8
NM = S // 128  # key tiles
F32 = mybir.dt.float32
F32R = mybir.dt.float32r
BF16 = mybir.dt.bfloat16
I16 = mybir.dt.int16
Alu = mybir.AluOpType
INV_SCALE = 1.0 / np.sqrt(HD)

# bf16 Schraudolph fast-exp constants (round-to-nearest variant).
_LOG2E = 1.4426950408889634
FE_A = float((1 << 7) * _LOG2E * INV_SCALE)  # folds the 1/sqrt(hd) scale
FE_B = float(127.0 * (1 << 7) - 8.25)

# which units use the fast path: heads with (h % 2 == 1), m in FAST_MS
FAST_MS = frozenset(range(NM))


def _is_fast(h, m):
    return (h % 2 == 1) and (m in FAST_MS)


def split_multiwaits(nc, max_waits=1):
    """This toolchain's walrus rejects >1 sync-wait per instruction; split
    extras into preceding single-wait NOPs on the same engine."""
    n = 0
    for f in nc.m.functions:
        for bb in f.blocks:
            out = []
            for ins in bb.instructions:
                si = ins.sync_info
                if (
                    si is not None
                    and si.on_wait is not None
                    and len(si.on_wait) > max_waits
                ):
                    waits = list(si.on_wait)
                    for j, w in enumerate(waits[:-max_waits]):
                        n += 1
                        out.append(
                            mybir.InstNoOp(
                                name=f"{ins.name}-wsplit{j}",
                                opcode="NoOp",
                                engine=ins.engine,
                                sync_info=mybir.SyncInfo(on_wait=[w], on_update=[]),
                            )
                        )
                    ins.sync_info = mybir.SyncInfo(
                        on_wait=waits[-max_waits:], on_update=list(si.on_update)
                    )
                out.append(ins)
            bb.instructions = out
    return n


def build_module():
    nc = bass.Bass("TRN2")
    xT_d = nc.dram_tensor("xT", [D, S], BF16, kind="ExternalInput")
    xTq_d = nc.dram_tensor("xTq", [D, SQ], BF16, kind="ExternalInput")
    # weights packed [128, 8*256]: (k0,k1,q0,q1,v0,v1,o0,o1)
    wpack_d = nc.dram_tensor("wpack", [128, 8 * D], BF16, kind="ExternalInput")
    # biases packed [128, 516]: bvb | bob | bq(2 cols) | bk(2 cols)
    bpack_d = nc.dram_tensor("bpack", [128, 2 * D + 4], F32, kind="ExternalInput")
    out_d = nc.dram_tensor("out", [SQ, D], F32, kind="ExternalOutput")

    with TileContext(nc) as tc:
        with (
            tc.tile_pool(name="const", bufs=1) as cpool,
            tc.tile_pool(name="pers", bufs=1) as pers,
        ):
            # ---- input DMAs, chunked + spread across engine queues ----
            xT_sb = [
                cpool.tile([128, S], BF16, tag=f"xT{d}", name=f"xT{d}")
                for d in range(2)
            ]
            xTq_sb = [
                cpool.tile([128, SQ], BF16, tag=f"xTq{d}", name=f"xTq{d}")
                for d in range(2)
            ]
            qs = [nc.sync, nc.scalar, nc.gpsimd]
            wpack_sb = cpool.tile([128, 8 * D], BF16, tag="wpack", name="wpack")
            bpack_sb = cpool.tile([128, 2 * D + 4], F32, tag="bpack", name="bpack")
            w_sb = {
                nm: [
                    wpack_sb[:, (2 * i + d) * D : (2 * i + d + 1) * D]
                    for d in range(2)
                ]
                for i, nm in enumerate(("k", "q", "v", "o"))
            }
            bvb_sb2 = bpack_sb[:, 0:D]
            bob_sb = bpack_sb[:, D : 2 * D]
            bq_sb2 = [bpack_sb[:, 2 * D + d : 2 * D + d + 1] for d in range(2)]
            bk_sb2 = [bpack_sb[:, 2 * D + 2 + d : 2 * D + 3 + d] for d in range(2)]
            jobs = [
                (wpack_sb[:, 0 : 6 * D], wpack_d[:, 0 : 6 * D]),
                (xT_sb[0][:, 0:512], xT_d[0:128, 0:512]),
                (xT_sb[1][:, 0:512], xT_d[128:256, 0:512]),
                (bpack_sb[:], bpack_d[:]),
                (xTq_sb[0][:], xTq_d[0:128, :]),
                (xTq_sb[1][:], xTq_d[128:256, :]),
                (wpack_sb[:, 6 * D : 8 * D], wpack_d[:, 6 * D : 8 * D]),
                (xT_sb[0][:, 512:2048], xT_d[0:128, 512:2048]),
                (xT_sb[1][:, 512:2048], xT_d[128:256, 512:2048]),
            ]
            for qi, (o, i_) in enumerate(jobs):
                qs[qi % 3].dma_start(out=o, in_=i_)

            # prime the exp activation table early (off the attention path)
            prime = cpool.tile([1, 1], F32, tag="prime", name="prime")
            nc.vector.memset(prime[:], 0.0)
            nc.scalar.activation(
                prime[:], prime[:], mybir.ActivationFunctionType.Exp
            )

            # ---- persistent intermediates ----
            KT_sb = [
                pers.tile([128, S], BF16, tag=f"KT{d}", name=f"KT{d}") for d in range(2)
            ]
            QTm_sb = [
                pers.tile([128, SQ], BF16, tag=f"QTm{h}", name=f"QTm{h}")
                for h in range(H)
            ]
            VA_sb = [
                pers.tile([128, H * (HD + 1)], F32R, tag=f"VA{m}", name=f"VA{m}")
                for m in range(NM)
            ]
            VAb_sb = [
                pers.tile([128, H * (HD + 1)], BF16, tag=f"VAb{m}", name=f"VAb{m}")
                for m in range(NM)
            ]
            ctxT_sb = [
                pers.tile([128, SQ], BF16, tag=f"ctxT{d}", name=f"ctxT{d}")
                for d in range(2)
            ]
            denS_sb = [
                pers.tile([1, SQ], F32, tag=f"denS{i}", name=f"denS{i}")
                for i in range(2)
            ]
            denT_sb = pers.tile([128, 16], F32, tag="denT", name="denT")
            recT_sb = pers.tile([128, 16], F32, tag="recT", name="recT")
            rb_sb = [
                pers.tile([HD, SQ], F32, tag=f"rb{i}", name=f"rb{i}") for i in range(2)
            ]

            # masked-Q tiles: zero everything once; Q bias-add writes in-head
            # rows. F32-packed memset path is ~10x faster than the bf16 one.
            for h in range(H):
                nc.vector.memset(QTm_sb[h][:].bitcast(F32), 0.0)
            # ones-columns for the denominator ride-along
            for m in range(NM):
                va = VA_sb[m][:].bitcast(F32).rearrange("p (h c) -> p h c", c=HD + 1)
                nc.vector.memset(va[:, :, HD : HD + 1], 1.0)

            # ---- projection helpers (psum tile passed in) ----
            def emit_k(half, kn, pkt):
                for d in range(2):
                    nc.tensor.matmul(
                        pkt[:],
                        lhsT=w_sb["k"][d][:, half * 128 : (half + 1) * 128],
                        rhs=xT_sb[d][:, kn * 512 : (kn + 1) * 512],
                        start=(d == 0),
                        stop=(d == 1),
                    )
                nc.vector.tensor_scalar_add(
                    KT_sb[half][:, kn * 512 : (kn + 1) * 512],
                    pkt[:],
                    bk_sb2[half],
                )

            def emit_q(half, qn, pqt):
                for d in range(2):
                    nc.tensor.matmul(
                        pqt[:],
                        lhsT=w_sb["q"][d][:, half * 128 : (half + 1) * 128],
                        rhs=xTq_sb[d][:, qn * 512 : (qn + 1) * 512],
                        start=(d == 0),
                        stop=(d == 1),
                    )
                for hh in range(4):
                    h = half * 4 + hh
                    nc.vector.tensor_scalar_add(
                        QTm_sb[h][
                            32 * hh : 32 * (hh + 1), qn * 512 : (qn + 1) * 512
                        ],
                        pqt[32 * hh : 32 * (hh + 1), :],
                        bq_sb2[half][32 * hh : 32 * (hh + 1), :],
                    )

            def emit_v(m, pvt):
                for d in range(2):
                    nc.tensor.matmul(
                        pvt[:],
                        lhsT=xT_sb[d][:, m * 128 : (m + 1) * 128],
                        rhs=w_sb["v"][d][:],
                        start=(d == 0),
                        stop=(d == 1),
                    )
                va = VA_sb[m][:].rearrange("p (h c) -> p h c", c=HD + 1)
                nc.vector.tensor_add(
                    va[:, :, 0:HD],
                    pvt[:].rearrange("p (h c) -> p h c", c=HD),
                    bvb_sb2.rearrange("p (h c) -> p h c", c=HD),
                )
                nc.gpsimd.tensor_copy(VAb_sb[m][:], VA_sb[m][:])

            # ---- early projections: K chunk 0, all Q, V m0-3 ----
            with tc.tile_pool(name="ps_kq", bufs=1, space="PSUM") as ps1:
                pksE = [
                    ps1.tile([128, 512], F32, tag=f"pk{i}", name=f"pk{i}")
                    for i in range(2)
                ]
                pqsE = [
                    ps1.tile([128, 512], F32, tag=f"pq{i}", name=f"pq{i}")
                    for i in range(2)
                ]
                pvsE = [
                    ps1.tile([128, D], F32, tag=f"pv{i}", name=f"pv{i}")
                    for i in range(2)
                ]
                emit_k(0, 0, pksE[0])
                emit_k(1, 0, pksE[1])
                emit_q(0, 0, pqsE[0])
                emit_q(0, 1, pqsE[1])
                emit_q(1, 0, pqsE[0])
                emit_q(1, 1, pqsE[1])
                for m in range(4):
                    emit_v(m, pvsE[m % 2])

            # ---- attention: pipelined units (head, m) ----
            with (
                tc.tile_pool(name="ps_att", bufs=1, space="PSUM") as psa,
                tc.tile_pool(name="etsA", bufs=4) as etsA,
                tc.tile_pool(name="etsB", bufs=4) as etsB,
                tc.tile_pool(name="dramp", bufs=2, space="DRAM") as dramp,
            ):
                pscs = [
                    psa.tile([128, SQ], F32, tag=f"psc{i}", name=f"psc{i}")
                    for i in range(2)
                ]
                pending = []  # deferred attn@V emissions, as (fn, args...)

                def emit_av(h, m, ets, pctx, base, fast):
                    vsb = VAb_sb[m] if fast else VA_sb[m]
                    for qn in range(2):
                        nc.tensor.matmul(
                            pctx[base : base + HD + 1, qn * 512 : (qn + 1) * 512],
                            lhsT=vsb[:, h * (HD + 1) : (h + 1) * (HD + 1)],
                            rhs=ets[:, qn * 512 : (qn + 1) * 512],
                            start=(m == 0),
                            stop=(m == NM - 1),
                        )

                fin_steps = []  # staged finalize thunks, one per unit

                def emit_finalize(hg, pctx):
                    # recip -> DRAM-bounce broadcast -> scale; staged so the
                    # engine-queue injections spread over several units
                    half = (2 * hg) // 4
                    rrec = dramp.tile([2, SQ], F32, tag="rrec", name=f"rrec{hg}")

                    def s1():
                        nc.scalar.activation(
                            denS_sb[0][:],
                            pctx[HD : HD + 1, :],
                            mybir.ActivationFunctionType.Copy,
                        )
                        nc.vector.tensor_copy(
                            denS_sb[1][:], pctx[64 + HD : 64 + HD + 1, :]
                        )
                        nc.sync.dma_start(out=denT_sb[0:64, :], in_=denS_sb[0][:])
                        nc.scalar.dma_start(out=denT_sb[64:128, :], in_=denS_sb[1][:])

                    def s2():
                        nc.vector.reciprocal(recT_sb[:], denT_sb[:])
                        nc.sync.dma_start(out=rrec[0:1, :], in_=recT_sb[0:64, :])
                        nc.scalar.dma_start(out=rrec[1:2, :], in_=recT_sb[64:128, :])
                        nc.sync.dma_start(
                            out=rb_sb[0][:], in_=rrec[0:1, :].to_broadcast((HD, SQ))
                        )
                        nc.scalar.dma_start(
                            out=rb_sb[1][:], in_=rrec[1:2, :].to_broadcast((HD, SQ))
                        )

                    def mk_mul(i, h):
                        def s():
                            base = 64 * i
                            hh = h % 4
                            nc.vector.tensor_mul(
                                ctxT_sb[half][32 * hh : 32 * (hh + 1), :],
                                pctx[base : base + HD, :],
                                rb_sb[i][:],
                            )

                        return s

                    fin_steps.extend(
                        [s1, s2, mk_mul(0, 2 * hg), mk_mul(1, 2 * hg + 1)]
                    )

                pop_ctr = [0]

                def pop_one():
                    h, m, ets, pctx_t, base, hg_of, fast = pending.pop(0)
                    emit_av(h, m, ets, pctx_t, base, fast)
                    if h % 2 == 1 and m == NM - 1:
                        emit_finalize(hg_of, pctx_t)
                    pop_ctr[0] += 1
                    if fin_steps and pop_ctr[0] % 2 == 0:
                        fin_steps.pop(0)()

                # pairs staggered by 8 m-tiles: two pairs in flight, two pctx
                # buffers; each pair's finalize has ~8 slots of slack before
                # its pctx banks are needed again
                STAG = 16
                units = []
                for s in range(STAG * (H // 2 - 1) + NM):
                    for g in range(H // 2):
                        m = s - STAG * g
                        if 0 <= m < NM:
                            for h in (2 * g, 2 * g + 1):
                                units.append((g, m, h))

                # late projections streamed into the early slots; their psum
                # banks hand over to pctx1 at slot 8
                pctx0_cm = tc.tile_pool(name="pctx0", bufs=1, space="PSUM")
                pctx0_pool = pctx0_cm.__enter__()
                late_cm = tc.tile_pool(name="ps_late", bufs=1, space="PSUM")
                late = late_cm.__enter__()
                pkL = late.tile([128, 512], F32, tag="pkL", name="pkL")
                pvL = late.tile([128, D], F32, tag="pvL", name="pvL")
                quota = []
                for kn in range(1, 4):
                    for mv in (4 * kn, 4 * kn + 1):
                        quota.append((emit_v, (mv, pvL)))
                    quota.append((emit_k, (0, kn, pkL)))
                    quota.append((emit_k, (1, kn, pkL)))
                    for mv in (4 * kn + 2, 4 * kn + 3):
                        quota.append((emit_v, (mv, pvL)))
                pctx1_cm = None
                pctx1_pool = None
                pctxs = {}
                uidx = 0
                for g, m, h in units:
                    slot = STAG * g + m
                    if slot >= STAG and late_cm is not None:
                        while quota:
                            fn, args = quota.pop(0)
                            fn(*args)
                        late_cm.__exit__(None, None, None)
                        late_cm = None
                        pctx1_cm = tc.tile_pool(
                            name="pctx1", bufs=1, space="PSUM"
                        )
                        pctx1_pool = pctx1_cm.__enter__()
                    if g not in pctxs:
                        pool_g = pctx0_pool if g % 2 == 0 else pctx1_pool
                        pctxs[g] = pool_g.tile(
                            [97, SQ], F32, tag="pctx", name=f"pctx{g}"
                        )
                    pctx = pctxs[g]
                    half = h // 4
                    psc = pscs[uidx % 2]
                    for qn in range(2):
                        nc.tensor.matmul(
                            psc[:, qn * 512 : (qn + 1) * 512],
                            lhsT=KT_sb[half][:, m * 128 : (m + 1) * 128],
                            rhs=QTm_sb[h][:, qn * 512 : (qn + 1) * 512],
                            start=True,
                            stop=True,
                        )
                    if _is_fast(h, m):
                        ets = etsB.tile([128, SQ], BF16, tag="eB", name="eB")
                        nc.vector.tensor_scalar(
                            ets[:].bitcast(I16),
                            psc[:],
                            FE_A,
                            FE_B,
                            Alu.mult,
                            Alu.add,
                        )
                    else:
                        ets = etsA.tile([128, SQ], F32R, tag="eA", name="eA")
                        nc.scalar.activation(
                            ets[:],
                            psc[:],
                            mybir.ActivationFunctionType.Exp,
                            scale=float(INV_SCALE),
                        )
                    base = 0 if h % 2 == 0 else 64
                    pending.append((h, m, ets, pctx, base, g, _is_fast(h, m)))
                    while len(pending) > 4:
                        pop_one()
                    for _ in range(2):
                        if quota:
                            fn, args = quota.pop(0)
                            fn(*args)
                    uidx += 1
                while pending:
                    pop_one()
                while fin_steps:
                    fin_steps.pop(0)()
                if pctx1_cm is not None:
                    pctx1_cm.__exit__(None, None, None)
                pctx0_cm.__exit__(None, None, None)

            # ---- output projection ----
            with (
                tc.tile_pool(name="ps_o", bufs=4, space="PSUM") as ps_o,
                tc.tile_pool(name="outp", bufs=4) as outp,
            ):
                for t in range(SQ // 128):
                    po = ps_o.tile([128, D], F32, tag="po", name="po")
                    for d in range(2):
                        nc.tensor.matmul(
                            po[:],
                            lhsT=ctxT_sb[d][:, t * 128 : (t + 1) * 128],
                            rhs=w_sb["o"][d][:],
                            start=(d == 0),
                            stop=(d == 1),
                        )
                    ot = outp.tile([128, D], F32, tag="ot", name="ot")
                    nc.vector.tensor_add(ot[:], po[:], bob_sb)
                    oq = nc.sync if t % 2 == 0 else nc.scalar
                    oq.dma_start(out=out_d[t * 128 : (t + 1) * 128, :], in_=ot[:])

    split_multiwaits(nc)
    return nc


_module_cache = {}


def _get_module():
    if "m" not in _module_cache:
        _module_cache["m"] = build_module()
    return _module_cache["m"]


def make_in_maps(inputs):
    x = np.asarray(inputs["x"], np.float32)
    Wq, bq = np.asarray(inputs["Wq"], np.float32), np.asarray(inputs["bq"], np.float32)
    Wk, bk = np.asarray(inputs["Wk"], np.float32), np.asarray(inputs["bk"], np.float32)
    Wv, bv = np.asarray(inputs["Wv"], np.float32), np.asarray(inputs["bv"], np.float32)
    Wo, bo = np.asarray(inputs["Wo"], np.float32), np.asarray(inputs["bo"], np.float32)
    hs = np.asarray(inputs["head_scale"], np.float32)

    import ml_dtypes

    bf16 = np.dtype(ml_dtypes.bfloat16)
    hs_col = np.repeat(hs, HD)  # head_scale folded into V
    wqT = Wq.T.astype(bf16)
    wkT = Wk.T.astype(bf16)
    wvT = (Wv * hs_col[:, None]).T.astype(bf16)
    woT = Wo.T.astype(bf16)
    wpack = np.concatenate(
        [wkT[0:128], wkT[128:256], wqT[0:128], wqT[128:256],
         wvT[0:128], wvT[128:256], woT[0:128], woT[128:256]],
        axis=1,
    )
    bpack = np.concatenate(
        [
            np.broadcast_to(bv * hs_col, (128, D)),
            np.broadcast_to(bo, (128, D)),
            bq.reshape(2, 128).T,
            bk.reshape(2, 128).T,
        ],
        axis=1,
    ).astype(np.float32)
    shared = {
        "wpack": np.ascontiguousarray(wpack),
        "bpack": np.ascontiguousarray(bpack),
    }
    in_maps = []
    for k in range(N_CORES):
        b, qh = k // 2, k % 2
        xT = np.ascontiguousarray(x[b].T.astype(bf16))
        in_maps.append(
            {
                "xT": xT,
                "xTq": np.ascontiguousarray(xT[:, qh * SQ : (qh + 1) * SQ]),
                **shared,
            }
        )
    return in_maps


def kernel(x, Wq, bq, Wk, bk, Wv, bv, Wo, bo, head_scale):
    in_maps = make_in_maps(
        dict(
            x=x, Wq=Wq, bq=bq, Wk=Wk, bk=bk, Wv=Wv, bv=bv, Wo=Wo, bo=bo,
            head_scale=head_scale,
        )
    )
    nc = _get_module()
    core_ids = list(range(N_CORES))
    # First execution after a fresh process attach has been observed to
    # return corrupted results on some cores; run once to warm up, then
    # use the second run (retry if it still looks corrupted).
    bass_utils.run_bass_kernel_spmd(nc, in_maps, core_ids=core_ids)
    for _ in range(3):
        res = bass_utils.run_bass_kernel_spmd(nc, in_maps, core_ids=core_ids)
        outs = [r["out"] for r in res.results]
        finite = all(np.isfinite(o).all() for o in outs)
        if finite and max(float(np.abs(o).max()) for o in outs) < 1e4:
            break
    full = np.stack(
        [np.concatenate([outs[2 * b], outs[2 * b + 1]], axis=0) for b in range(B)]
    )
    return full.astype(np.float32)
